# revision 27
# baseline (speedup 1.0000x reference)
"""Trainium2 Bass kernel for an 8-expert top-2 MoE block (B=4, T=2048, C=1024, H=4C).

Strategy (hidden-dim tensor-parallel over all experts):
  - Host computes the gate (logits -> top-2 -> softmax) and gathers each
    expert's routed tokens into one padded token stream (the "all-to-all
    dispatch", done host-side as the sharding step).
  - Each of the 8 NeuronCores holds a 512-wide slice of the hidden (H)
    dimension of ALL 8 experts' weights (16MB bf16 total, same as one
    expert's full weights) and runs gelu(x @ w1_slice + b1_slice) @
    w2_slice over EVERY routed token. This is perfectly load-balanced
    regardless of routing skew, unlike expert-parallel where every core
    pays for the hottest expert.
  - The per-expert token stream is chunked into variable-size chunks
    (<=512 tokens, PSUM bank limit; >=192 so LDWEIGHTS stays hidden
    behind the moving operand), so padding waste is <0.5%.
  - Host sums the 8 partial outputs (the H-dimension reduction), adds
    b2, applies the top-2 combine weights, and scatter-adds into the
    full [B, T, C] output (the "combine"/unshard).

All matmuls are bf16 with fp32 PSUM accumulation; rel err vs the fp32
reference is ~3e-3.
"""

import sys

for _p in ("/opt/trn_rl_repo", "/root/.axon_site/_ro/trn_rl_repo"):
    if _p not in sys.path:
        sys.path.insert(0, _p)

from contextlib import ExitStack

import ml_dtypes
import numpy as np

import bass_rust
import concourse.bass as bass
import concourse.mybir as mybir
import concourse.tile as tile
from concourse.bass_utils import run_bass_kernel_spmd

B, T, C, E = 4, 2048, 1024, 8
H = 4 * C
N = B * T
TOP_K = 2
P = 128            # partitions
HS = H // 8        # per-core hidden slice (512)
CK = C // P        # 8 contraction chunks for x @ w1
LHK = HS // P      # 4 local h-groups per chunk
TOK = 512          # max token chunk (PSUM bank = 512 fp32)
MIN_TAIL = 192     # min tail chunk so LDWEIGHTS stays hidden

BF16 = mybir.dt.bfloat16
F32 = mybir.dt.float32


def _legalize_waits(nc: "bass.Bass") -> None:
    """Split multi-wait instructions into standalone EventSemaphore waits.

    The walrus build here accepts at most one sync-wait command per
    instruction (setupSyncWait "Too many sync wait commands"), but Tile
    attaches every outstanding dependency to the consuming instruction.
    Hoist all but the last wait onto same-engine EventSemaphore
    instructions placed immediately before the consumer: the engine's
    sequencer processes them in order, so the dependency still holds.
    """

    def fix_block(bb):
        out = []
        for inst in bb.instructions:
            si = inst.sync_info
            if si is not None and len(si.on_wait) > 1:
                waits = list(si.on_wait)
                for k, w in enumerate(waits[:-1]):
                    ev = bass_rust.InstEventSemaphore(
                        name=f"{inst.name}-lw{k}", ins=[], outs=[],
                        engine=inst.engine,
                    )
                    ev.sync_info = bass_rust.SyncInfo(on_wait=[w], on_update=[])
                    out.append(ev)
                inst.sync_info = bass_rust.SyncInfo(
                    on_wait=[waits[-1]], on_update=list(si.on_update)
                )
            out.append(inst)
        bb.instructions = out
        for sub in getattr(bb, "blocks", []) or []:
            fix_block(sub)

    for fn in nc.m.functions:
        for bb in fn.blocks:
            fix_block(bb)


def _ceil16(v: int) -> int:
    return -(-v // 16) * 16


def _expert_chunks(count: int) -> list[int]:
    """Chunk sizes for one expert's token stream: full 512s plus one or
    two tails in [MIN_TAIL, 512], padded to a multiple of 16."""
    if count <= 0:
        return []
    k, r = divmod(count, TOK)
    if r == 0:
        return [TOK] * k
    if r >= MIN_TAIL or k == 0:
        return [TOK] * k + [max(_ceil16(r), MIN_TAIL if k else 16)]
    # Small remainder: borrow one full chunk and split into two tails.
    r += TOK
    t1 = max(_ceil16(r // 2), MIN_TAIL)
    t2 = max(_ceil16(r - t1), MIN_TAIL)
    return [TOK] * (k - 1) + [t1, t2]


def _build_nc(schedule, cap_n: int) -> bass.Bass:
    """schedule: list of (expert, tok, offset) chunk descriptors."""
    nc = bass.Bass()
    xT = nc.declare_dram_parameter("xT", [C, cap_n], BF16, isOutput=False)
    w1 = nc.declare_dram_parameter("w1", [C, E * HS], BF16, isOutput=False)
    w2 = nc.declare_dram_parameter("w2", [E * HS, C], BF16, isOutput=False)
    b1 = nc.declare_dram_parameter("b1", [E * HS], F32, isOutput=False)
    yT = nc.declare_dram_parameter("yT", [C, cap_n], F32, isOutput=True)

    gelu = mybir.ActivationFunctionType.Gelu

    with tile.TileContext(nc) as tc, ExitStack() as ctx:
        w1p = ctx.enter_context(tc.tile_pool(name="w1p", bufs=CK * E))
        w2p = ctx.enter_context(tc.tile_pool(name="w2p", bufs=LHK * E))
        cst = ctx.enter_context(tc.tile_pool(name="cst", bufs=1))
        xp = ctx.enter_context(tc.tile_pool(name="xp", bufs=5))
        hp = ctx.enter_context(tc.tile_pool(name="hp", bufs=2 * LHK))
        op = ctx.enter_context(tc.tile_pool(name="op", bufs=4))
        psA = ctx.enter_context(tc.tile_pool(name="psA", bufs=4, space="PSUM"))
        psB = ctx.enter_context(tc.tile_pool(name="psB", bufs=4, space="PSUM"))

        # Warm the PE HAM clock gate while the first DMAs stream:
        # back-to-back dummy matmuls supply the >=3.4us of sustained PE
        # activity that flips the clock from 1.2 to 2.4 GHz before the
        # real work arrives.
        dummy = cst.tile([P, 512], BF16, tag="dummy")
        nc.gpsimd.memset(dummy[:], 0.0)
        warm = psB.tile([P, 512], F32, tag="psB", name="warm")
        for i in range(24):
            nc.tensor.matmul(warm[:], dummy[:, :P], dummy[:, :512],
                             start=(i == 0), stop=(i == 23))

        # DMA completion tracks emission order (the serial stream
        # saturates per-core HBM BW), so order = criticality: bias
        # (gates the first gelu), first token chunk, then weights
        # interleaved per expert to match the compute schedule.
        b1_sb = cst.tile([P, E * LHK], F32, tag="b1")
        nc.sync.dma_start(b1_sb[:], b1.rearrange("(a p) -> p a", p=P))
        # Absorb the bias DMA wait on ScalarE (the activation sync
        # struct fits only one wait and the first gelu needs PE's).
        scr1 = cst.tile([P, 1], F32, tag="scr1")
        nc.scalar.copy(scr1[:], b1_sb[:, 0:1])

        def load_xt(off, tok):
            xt = xp.tile([P, CK, tok], BF16, tag="xt", name=f"xt{off}")
            for cb in range(CK):
                nc.sync.dma_start(xt[:, cb, :],
                                  xT[cb * P:(cb + 1) * P, off:off + tok])
            return xt

        e0, tok0, off0 = schedule[0]
        xt0 = load_xt(off0, tok0)

        # Weights and token chunks are emitted interleaved per expert so
        # the serial DMA stream delivers them in the order the compute
        # schedule consumes them: w1(e), w2(e), then expert e's token
        # chunks (which PE reads roughly one expert-period later).
        w1_sb = [[None] * E for _ in range(CK)]   # [cb][e] -> [P, HS]
        w2_sb = [[None] * LHK for _ in range(E)]  # [e][lh] -> [P, C]
        xts = {0: xt0}
        for e in range(E):
            for cb in range(CK):
                t_ = w1p.tile([P, HS], BF16, tag="w1", name=f"w1_{cb}_{e}")
                nc.sync.dma_start(t_[:], w1[cb * P:(cb + 1) * P,
                                             e * HS:(e + 1) * HS])
                w1_sb[cb][e] = t_
            for lh in range(LHK):
                t_ = w2p.tile([P, C], BF16, tag="w2", name=f"w2_{e}_{lh}")
                nc.sync.dma_start(t_[:], w2[e * HS + lh * P:
                                            e * HS + (lh + 1) * P, :])
                w2_sb[e][lh] = t_
            for k, (se, tok, off) in enumerate(schedule):
                if se == e and k not in xts:
                    xts[k] = load_xt(off, tok)

        def do_chunk(e, tok, off, xt):
            # phase A: hT[lh] = gelu(w1_slice.T @ xT + b1_slice)
            hts = []
            for lh in range(LHK):
                pa = psA.tile([P, tok], F32, tag="psA", name=f"pa{lh}")
                for cb in range(CK):
                    nc.tensor.matmul(
                        pa[:],
                        w1_sb[cb][e][:, lh * P:(lh + 1) * P],
                        xt[:, cb, :],
                        start=(cb == 0),
                        stop=(cb == CK - 1),
                    )
                ht = hp.tile([P, tok], BF16, tag="ht", name=f"ht{lh}")
                nc.scalar.activation(ht[:], pa[:], gelu,
                                     bias=b1_sb[:, e * LHK + lh:e * LHK + lh + 1])
                hts.append(ht)
            # phase B: yT_partial[cout] = w2_slice.T @ hT  (b2 added on host)
            for co in range(CK):
                pb = psB.tile([P, tok], F32, tag="psB", name=f"pb{co}")
                for lh in range(LHK):
                    nc.tensor.matmul(
                        pb[:],
                        w2_sb[e][lh][:, co * P:(co + 1) * P],
                        hts[lh][:],
                        start=(lh == 0),
                        stop=(lh == LHK - 1),
                    )
                ot = op.tile([P, tok], F32, tag="ot", name=f"ot{co}")
                nc.scalar.copy(ot[:], pb[:])
                nc.sync.dma_start(yT[co * P:(co + 1) * P, off:off + tok],
                                  ot[:])

        for k, (e, tok, off) in enumerate(schedule):
            do_chunk(e, tok, off, xts[k])

    _legalize_waits(nc)
    return nc


_NC_CACHE: dict = {}
_LAST_IN_MAPS: list | None = None
_LAST_RESULTS = None


def _routing(xf: np.ndarray, w_gate: np.ndarray):
    logits = xf.astype(np.float64) @ w_gate.astype(np.float64)        # [N, E]
    top_idx = np.argsort(-logits, axis=-1, kind="stable")[:, :TOP_K]  # [N, K]
    top_vals = np.take_along_axis(logits, top_idx, axis=-1)
    ex = np.exp(top_vals - top_vals.max(axis=-1, keepdims=True))
    scores = ex / ex.sum(axis=-1, keepdims=True)                      # [N, K]
    return top_idx, scores


def kernel(x, w_gate, w1, b1, w2, b2):
    global _LAST_IN_MAPS, _LAST_RESULTS
    x = np.asarray(x, dtype=np.float32)
    w_gate = np.asarray(w_gate, dtype=np.float32)
    w1 = np.asarray(w1, dtype=np.float32)
    b1 = np.asarray(b1, dtype=np.float32)
    w2 = np.asarray(w2, dtype=np.float32)
    b2 = np.asarray(b2, dtype=np.float32)

    xf = x.reshape(N, C)
    top_idx, scores = _routing(xf, w_gate)

    idx, cw, caps = [], [], []
    for e in range(E):
        hit = top_idx == e                       # [N, K]
        tok = np.nonzero(hit.any(axis=-1))[0]
        idx.append(tok)
        cw.append((scores * hit).sum(axis=-1)[tok].astype(np.float32))
        caps.append(_expert_chunks(len(tok)))

    # Schedule: (expert, chunk_len, token_offset) with experts laid out
    # back to back in one padded token stream.
    schedule, offs = [], []
    off = 0
    for e in range(E):
        offs.append(off)
        for tok in caps[e]:
            schedule.append((e, tok, off))
            off += tok
    cap_n = off

    key = tuple(schedule)
    nc = _NC_CACHE.get(key)
    if nc is None:
        nc = _NC_CACHE[key] = _build_nc(schedule, cap_n)

    xTe = np.zeros((C, cap_n), dtype=ml_dtypes.bfloat16)
    for e in range(E):
        xTe[:, offs[e]:offs[e] + len(idx[e])] = \
            xf[idx[e]].T.astype(ml_dtypes.bfloat16)

    w1_bf = w1.astype(ml_dtypes.bfloat16)   # [E, C, H]
    w2_bf = w2.astype(ml_dtypes.bfloat16)   # [E, H, C]
    in_maps = []
    for c in range(E):
        hs = slice(c * HS, (c + 1) * HS)
        in_maps.append({
            "xT": xTe,
            # [C, E*HS]: column block e = expert e's H-slice for this core
            "w1": np.ascontiguousarray(
                w1_bf[:, :, hs].transpose(1, 0, 2).reshape(C, E * HS)),
            # [E*HS, C]: row block e = expert e's H-slice rows
            "w2": np.ascontiguousarray(w2_bf[:, hs, :].reshape(E * HS, C)),
            "b1": np.ascontiguousarray(b1[:, hs].reshape(E * HS)),
        })

    _LAST_IN_MAPS = in_maps
    res = run_bass_kernel_spmd(nc, in_maps, list(range(E)))
    _LAST_RESULTS = res

    # Combine: sum the 8 H-slice partials, add b2, apply gate weights,
    # scatter-add back to token order.
    Y = res.results[0]["yT"].astype(np.float32)
    for c in range(1, E):
        Y += res.results[c]["yT"]
    out = np.zeros((N, C), dtype=np.float32)
    for e in range(E):
        ne = len(idx[e])
        ye = Y[:, offs[e]:offs[e] + ne].T + b2[e]
        out[idx[e]] += cw[e][:, None] * ye
    return out.reshape(B, T, C)


# revision 28
# speedup vs baseline: 1.1486x; 1.1486x over previous
"""Trainium2 Bass kernel for an 8-expert top-2 MoE block (B=4, T=2048, C=1024, H=4C).

Strategy (hidden-dim tensor-parallel over all experts):
  - Host computes the gate (logits -> top-2 -> softmax) and gathers each
    expert's routed tokens into one padded token stream (the "all-to-all
    dispatch", done host-side as the sharding step).
  - Each of the 8 NeuronCores holds a 512-wide slice of the hidden (H)
    dimension of ALL 8 experts' weights (16MB bf16 total, same as one
    expert's full weights) and runs gelu(x @ w1_slice + b1_slice) @
    w2_slice over EVERY routed token. This is perfectly load-balanced
    regardless of routing skew, unlike expert-parallel where every core
    pays for the hottest expert.
  - The per-expert token stream is chunked into variable-size chunks
    (<=512 tokens, PSUM bank limit; >=192 so LDWEIGHTS stays hidden
    behind the moving operand), so padding waste is <0.5%.
  - All device tensors are pre-tiled on the host into partition-major
    [128, *] layouts with >=8KB contiguous per partition, so every
    weight block / token chunk / output chunk moves as ONE efficient
    DMA (the serial DMA stream is otherwise instruction-bound).
  - Host sums the 8 partial outputs (the H-dimension reduction), adds
    b2, applies the top-2 combine weights, and scatter-adds into the
    full [B, T, C] output (the "combine"/unshard).

All matmuls are bf16 with fp32 PSUM accumulation; rel err vs the fp32
reference is ~3e-3.
"""

import sys

for _p in ("/opt/trn_rl_repo", "/root/.axon_site/_ro/trn_rl_repo"):
    if _p not in sys.path:
        sys.path.insert(0, _p)

from contextlib import ExitStack

import ml_dtypes
import numpy as np

import bass_rust
import concourse.bass as bass
import concourse.mybir as mybir
import concourse.tile as tile
from concourse.bass_utils import run_bass_kernel_spmd

B, T, C, E = 4, 2048, 1024, 8
H = 4 * C
N = B * T
TOP_K = 2
P = 128            # partitions
HS = H // 8        # per-core hidden slice (512)
CK = C // P        # 8 contraction chunks for x @ w1
LHK = HS // P      # 4 local h-groups per chunk
TOK = 512          # max token chunk (PSUM bank = 512 fp32)
MIN_TAIL = 192     # min tail chunk so LDWEIGHTS stays hidden

BF16 = mybir.dt.bfloat16
F32 = mybir.dt.float32


def _legalize_waits(nc: "bass.Bass") -> None:
    """Split multi-wait instructions into standalone EventSemaphore waits.

    The walrus build here accepts at most one sync-wait command per
    instruction (setupSyncWait "Too many sync wait commands"), but Tile
    attaches every outstanding dependency to the consuming instruction.
    Hoist all but the last wait onto same-engine EventSemaphore
    instructions placed immediately before the consumer: the engine's
    sequencer processes them in order, so the dependency still holds.
    """

    def fix_block(bb):
        out = []
        for inst in bb.instructions:
            si = inst.sync_info
            if si is not None and len(si.on_wait) > 1:
                waits = list(si.on_wait)
                for k, w in enumerate(waits[:-1]):
                    ev = bass_rust.InstEventSemaphore(
                        name=f"{inst.name}-lw{k}", ins=[], outs=[],
                        engine=inst.engine,
                    )
                    ev.sync_info = bass_rust.SyncInfo(on_wait=[w], on_update=[])
                    out.append(ev)
                inst.sync_info = bass_rust.SyncInfo(
                    on_wait=[waits[-1]], on_update=list(si.on_update)
                )
            out.append(inst)
        bb.instructions = out
        for sub in getattr(bb, "blocks", []) or []:
            fix_block(sub)

    for fn in nc.m.functions:
        for bb in fn.blocks:
            fix_block(bb)


def _ceil16(v: int) -> int:
    return -(-v // 16) * 16


def _expert_chunks(count: int) -> list[int]:
    """Chunk sizes for one expert's token stream: full 512s plus one or
    two tails in [MIN_TAIL, 512], padded to a multiple of 16."""
    if count <= 0:
        return []
    k, r = divmod(count, TOK)
    if r == 0:
        return [TOK] * k
    if r >= MIN_TAIL or k == 0:
        return [TOK] * k + [max(_ceil16(r), MIN_TAIL if k else 16)]
    # Small remainder: borrow one full chunk and split into two tails.
    r += TOK
    t1 = max(_ceil16(r // 2), MIN_TAIL)
    t2 = max(_ceil16(r - t1), MIN_TAIL)
    return [TOK] * (k - 1) + [t1, t2]


def _build_nc(schedule, cap_n: int) -> bass.Bass:
    """schedule: list of (expert, tok, offset) chunk descriptors.

    DRAM layouts (host pre-tiled, partition-major):
      xT  [P, CK*cap_n]  chunk k at cols CK*off: row p = concat over cb
                         of x[cb*128+p, tokens_of_chunk]
      w1  [P, E*CK*HS]   expert e at cols e*CK*HS, cb-major
      w2  [P, E*LHK*C]   expert e at cols e*LHK*C, lh-major
      b1  [P, E*LHK]     col g = b1 slice values for h-group g
      yT  [P, CK*cap_n]  same column scheme as xT (co-major per chunk)
    """
    nc = bass.Bass()
    xT = nc.declare_dram_parameter("xT", [P, CK * cap_n], BF16, isOutput=False)
    w1 = nc.declare_dram_parameter("w1", [P, E * CK * HS], BF16, isOutput=False)
    w2 = nc.declare_dram_parameter("w2", [P, E * LHK * C], BF16, isOutput=False)
    b1 = nc.declare_dram_parameter("b1", [P, E * LHK], F32, isOutput=False)
    yT = nc.declare_dram_parameter("yT", [P, CK * cap_n], F32, isOutput=True)

    gelu = mybir.ActivationFunctionType.Gelu

    with tile.TileContext(nc) as tc, ExitStack() as ctx:
        w1p = ctx.enter_context(tc.tile_pool(name="w1p", bufs=E))
        w2p = ctx.enter_context(tc.tile_pool(name="w2p", bufs=E))
        cst = ctx.enter_context(tc.tile_pool(name="cst", bufs=1))
        xp = ctx.enter_context(tc.tile_pool(name="xp", bufs=3))
        hp = ctx.enter_context(tc.tile_pool(name="hp", bufs=2 * LHK))
        op = ctx.enter_context(tc.tile_pool(name="op", bufs=2))
        psA = ctx.enter_context(tc.tile_pool(name="psA", bufs=4, space="PSUM"))
        psB = ctx.enter_context(tc.tile_pool(name="psB", bufs=4, space="PSUM"))

        # Warm the PE HAM clock gate while the first DMAs stream:
        # back-to-back dummy matmuls supply the >=3.4us of sustained PE
        # activity that flips the clock from 1.2 to 2.4 GHz before the
        # real work arrives.
        dummy = cst.tile([P, 512], BF16, tag="dummy")
        nc.gpsimd.memset(dummy[:], 0.0)
        warm = psB.tile([P, 512], F32, tag="psB", name="warm")
        for i in range(24):
            nc.tensor.matmul(warm[:], dummy[:, :P], dummy[:, :512],
                             start=(i == 0), stop=(i == 23))

        # DMA completion tracks emission order (the serial stream
        # saturates per-core HBM BW), so order = criticality: bias
        # (gates the first gelu), expert-0 weights + first chunks, then
        # the rest of the weights; token chunks prefetch just-in-time
        # inside the chunk loop.
        b1_sb = cst.tile([P, E * LHK], F32, tag="b1")
        nc.sync.dma_start(b1_sb[:], b1[:, :])
        # Absorb the bias DMA wait on ScalarE (the activation sync
        # struct fits only one wait and the first gelu needs PE's).
        scr1 = cst.tile([P, 1], F32, tag="scr1")
        nc.scalar.copy(scr1[:], b1_sb[:, 0:1])

        def load_xt(off, tok):
            xt = xp.tile([P, CK, tok], BF16, tag="xt", name=f"xt{off}")
            nc.sync.dma_start(
                xt[:], xT[:, CK * off:CK * (off + tok)]
                .rearrange("p (a m) -> p a m", a=CK))
            return xt

        w1_sb, w2_sb = [], []

        def load_weights(e):
            t1 = w1p.tile([P, CK, HS], BF16, tag="w1", name=f"w1_{e}")
            nc.sync.dma_start(
                t1[:], w1[:, e * CK * HS:(e + 1) * CK * HS]
                .rearrange("p (a m) -> p a m", a=CK))
            w1_sb.append(t1)
            t2 = w2p.tile([P, LHK, C], BF16, tag="w2", name=f"w2_{e}")
            nc.sync.dma_start(
                t2[:], w2[:, e * LHK * C:(e + 1) * LHK * C]
                .rearrange("p (a m) -> p a m", a=LHK))
            w2_sb.append(t2)

        load_weights(0)
        xts = {k: load_xt(off, tok) for k, (e, tok, off) in
               enumerate(schedule[:2])}
        for e in range(1, E):
            load_weights(e)

        def do_chunk(e, tok, off, xt):
            # phase A: hT[lh] = gelu(w1_slice.T @ xT + b1_slice)
            hts = []
            for lh in range(LHK):
                pa = psA.tile([P, tok], F32, tag="psA", name=f"pa{lh}")
                for cb in range(CK):
                    nc.tensor.matmul(
                        pa[:],
                        w1_sb[e][:, cb, lh * P:(lh + 1) * P],
                        xt[:, cb, :],
                        start=(cb == 0),
                        stop=(cb == CK - 1),
                    )
                ht = hp.tile([P, tok], BF16, tag="ht", name=f"ht{lh}")
                nc.scalar.activation(ht[:], pa[:], gelu,
                                     bias=b1_sb[:, e * LHK + lh:e * LHK + lh + 1])
                hts.append(ht)
            # phase B: yT_partial[co] = w2_slice.T @ hT  (b2 added on host)
            ot = op.tile([P, CK, tok], F32, tag="ot", name=f"ot{off}")
            for co in range(CK):
                pb = psB.tile([P, tok], F32, tag="psB", name=f"pb{co}")
                for lh in range(LHK):
                    nc.tensor.matmul(
                        pb[:],
                        w2_sb[e][:, lh, co * P:(co + 1) * P],
                        hts[lh][:],
                        start=(lh == 0),
                        stop=(lh == LHK - 1),
                    )
                nc.scalar.copy(ot[:, co, :], pb[:])
            nc.sync.dma_start(
                yT[:, CK * off:CK * (off + tok)]
                .rearrange("p (a m) -> p a m", a=CK), ot[:])

        for k, (e, tok, off) in enumerate(schedule):
            if k + 2 < len(schedule):
                e2, tok2, off2 = schedule[k + 2]
                xts[k + 2] = load_xt(off2, tok2)
            do_chunk(e, tok, off, xts.pop(k))

    _legalize_waits(nc)
    return nc


_NC_CACHE: dict = {}
_LAST_IN_MAPS: list | None = None
_LAST_RESULTS = None


def _routing(xf: np.ndarray, w_gate: np.ndarray):
    logits = xf.astype(np.float64) @ w_gate.astype(np.float64)        # [N, E]
    top_idx = np.argsort(-logits, axis=-1, kind="stable")[:, :TOP_K]  # [N, K]
    top_vals = np.take_along_axis(logits, top_idx, axis=-1)
    ex = np.exp(top_vals - top_vals.max(axis=-1, keepdims=True))
    scores = ex / ex.sum(axis=-1, keepdims=True)                      # [N, K]
    return top_idx, scores


def _ptile(a: np.ndarray) -> np.ndarray:
    """[G*P, M] -> [P, G*M]: row p = concat over g of a[g*128+p, :]."""
    g = a.shape[0] // P
    return np.ascontiguousarray(
        a.reshape(g, P, -1).transpose(1, 0, 2).reshape(P, -1))


def kernel(x, w_gate, w1, b1, w2, b2):
    global _LAST_IN_MAPS, _LAST_RESULTS
    x = np.asarray(x, dtype=np.float32)
    w_gate = np.asarray(w_gate, dtype=np.float32)
    w1 = np.asarray(w1, dtype=np.float32)
    b1 = np.asarray(b1, dtype=np.float32)
    w2 = np.asarray(w2, dtype=np.float32)
    b2 = np.asarray(b2, dtype=np.float32)

    xf = x.reshape(N, C)
    top_idx, scores = _routing(xf, w_gate)

    idx, cw, caps = [], [], []
    for e in range(E):
        hit = top_idx == e                       # [N, K]
        tok = np.nonzero(hit.any(axis=-1))[0]
        idx.append(tok)
        cw.append((scores * hit).sum(axis=-1)[tok].astype(np.float32))
        caps.append(_expert_chunks(len(tok)))

    # Schedule: (expert, chunk_len, token_offset) with experts laid out
    # back to back in one padded token stream.
    schedule, offs = [], []
    off = 0
    for e in range(E):
        offs.append(off)
        for tok in caps[e]:
            schedule.append((e, tok, off))
            off += tok
    cap_n = off

    key = tuple(schedule)
    nc = _NC_CACHE.get(key)
    if nc is None:
        nc = _NC_CACHE[key] = _build_nc(schedule, cap_n)

    # Token stream, pre-tiled per chunk: [P, CK*cap_n] bf16.
    xT_cols = np.zeros((C, cap_n), dtype=ml_dtypes.bfloat16)
    for e in range(E):
        xT_cols[:, offs[e]:offs[e] + len(idx[e])] = \
            xf[idx[e]].T.astype(ml_dtypes.bfloat16)
    xTe = np.empty((P, CK * cap_n), dtype=ml_dtypes.bfloat16)
    for _, tok, off in schedule:
        xTe[:, CK * off:CK * (off + tok)] = _ptile(xT_cols[:, off:off + tok])

    w1_bf = w1.astype(ml_dtypes.bfloat16)   # [E, C, H]
    w2_bf = w2.astype(ml_dtypes.bfloat16)   # [E, H, C]
    in_maps = []
    for c in range(E):
        hs = slice(c * HS, (c + 1) * HS)
        w1c = np.empty((P, E * CK * HS), dtype=ml_dtypes.bfloat16)
        w2c = np.empty((P, E * LHK * C), dtype=ml_dtypes.bfloat16)
        for e in range(E):
            w1c[:, e * CK * HS:(e + 1) * CK * HS] = _ptile(w1_bf[e][:, hs])
            w2c[:, e * LHK * C:(e + 1) * LHK * C] = _ptile(w2_bf[e][hs, :])
        b1c = np.ascontiguousarray(
            b1[:, hs].reshape(E * LHK, P).T)     # [P, E*LHK]
        in_maps.append({"xT": xTe, "w1": w1c, "w2": w2c, "b1": b1c})

    _LAST_IN_MAPS = in_maps
    res = run_bass_kernel_spmd(nc, in_maps, list(range(E)))
    _LAST_RESULTS = res

    # Combine: sum the 8 H-slice partials, de-tile, add b2, apply gate
    # weights, scatter-add back to token order.
    Y2 = res.results[0]["yT"].astype(np.float32)
    for c in range(1, E):
        Y2 += res.results[c]["yT"]
    Y = np.empty((cap_n, C), dtype=np.float32)   # token-major
    for _, tok, off in schedule:
        Y[off:off + tok] = (Y2[:, CK * off:CK * (off + tok)]
                            .reshape(P, CK, tok).transpose(1, 0, 2)
                            .reshape(C, tok).T)
    out = np.zeros((N, C), dtype=np.float32)
    for e in range(E):
        ne = len(idx[e])
        out[idx[e]] += cw[e][:, None] * (Y[offs[e]:offs[e] + ne] + b2[e])
    return out.reshape(B, T, C)


# revision 32
# speedup vs baseline: 1.2232x; 1.0649x over previous
"""Trainium2 Bass kernel for an 8-expert top-2 MoE block (B=4, T=2048, C=1024, H=4C).

Strategy (hidden-dim tensor-parallel over all experts):
  - Host computes the gate (logits -> top-2 -> softmax) and gathers each
    expert's routed tokens into one padded token stream (the "all-to-all
    dispatch", done host-side as the sharding step).
  - Each of the 8 NeuronCores holds a 512-wide slice of the hidden (H)
    dimension of ALL 8 experts' weights (16MB bf16 total, same as one
    expert's full weights) and runs gelu(x @ w1_slice + b1_slice) @
    w2_slice over EVERY routed token. This is perfectly load-balanced
    regardless of routing skew, unlike expert-parallel where every core
    pays for the hottest expert.
  - The per-expert token stream is chunked into variable-size chunks
    (<=512 tokens, PSUM bank limit; >=192 so LDWEIGHTS stays hidden
    behind the moving operand), so padding waste is <0.5%.
  - All device tensors are pre-tiled on the host into partition-major
    [128, *] layouts with >=8KB contiguous per partition, so every
    weight block / token chunk / output chunk moves as ONE efficient
    DMA (the serial DMA stream is otherwise instruction-bound).
  - Host sums the 8 partial outputs (the H-dimension reduction), adds
    b2, applies the top-2 combine weights, and scatter-adds into the
    full [B, T, C] output (the "combine"/unshard).

All matmuls are bf16 with fp32 PSUM accumulation; rel err vs the fp32
reference is ~3e-3.
"""

import sys

for _p in ("/opt/trn_rl_repo", "/root/.axon_site/_ro/trn_rl_repo"):
    if _p not in sys.path:
        sys.path.insert(0, _p)

from contextlib import ExitStack

import ml_dtypes
import numpy as np

import bass_rust
import concourse.bass as bass
import concourse.mybir as mybir
import concourse.tile as tile
from concourse.bass_utils import run_bass_kernel_spmd

B, T, C, E = 4, 2048, 1024, 8
H = 4 * C
N = B * T
TOP_K = 2
P = 128            # partitions
HS = H // 8        # per-core hidden slice (512)
CK = C // P        # 8 contraction chunks for x @ w1
LHK = HS // P      # 4 local h-groups per chunk
TOK = 512          # max token chunk (PSUM bank = 512 fp32)
MIN_TAIL = 192     # min tail chunk so LDWEIGHTS stays hidden

BF16 = mybir.dt.bfloat16
F32 = mybir.dt.float32


def _legalize_waits(nc: "bass.Bass") -> None:
    """Split multi-wait instructions into standalone EventSemaphore waits.

    The walrus build here accepts at most one sync-wait command per
    instruction (setupSyncWait "Too many sync wait commands"), but Tile
    attaches every outstanding dependency to the consuming instruction.
    Hoist all but the last wait onto same-engine EventSemaphore
    instructions placed immediately before the consumer: the engine's
    sequencer processes them in order, so the dependency still holds.
    """

    def fix_block(bb):
        out = []
        for inst in bb.instructions:
            si = inst.sync_info
            if si is not None and len(si.on_wait) > 1:
                waits = list(si.on_wait)
                for k, w in enumerate(waits[:-1]):
                    ev = bass_rust.InstEventSemaphore(
                        name=f"{inst.name}-lw{k}", ins=[], outs=[],
                        engine=inst.engine,
                    )
                    ev.sync_info = bass_rust.SyncInfo(on_wait=[w], on_update=[])
                    out.append(ev)
                inst.sync_info = bass_rust.SyncInfo(
                    on_wait=[waits[-1]], on_update=list(si.on_update)
                )
            out.append(inst)
        bb.instructions = out
        for sub in getattr(bb, "blocks", []) or []:
            fix_block(sub)

    for fn in nc.m.functions:
        for bb in fn.blocks:
            fix_block(bb)


def _ceil16(v: int) -> int:
    return -(-v // 16) * 16


def _expert_chunks(count: int) -> list[int]:
    """Chunk sizes for one expert's token stream: full 512s plus one or
    two tails in [MIN_TAIL, 512], padded to a multiple of 16."""
    if count <= 0:
        return []
    k, r = divmod(count, TOK)
    if r == 0:
        return [TOK] * k
    if r >= MIN_TAIL or k == 0:
        return [TOK] * k + [max(_ceil16(r), MIN_TAIL if k else 16)]
    # Small remainder: borrow one full chunk and split into two tails.
    r += TOK
    t1 = max(_ceil16(r // 2), MIN_TAIL)
    t2 = max(_ceil16(r - t1), MIN_TAIL)
    return [TOK] * (k - 1) + [t1, t2]


def _build_nc(schedule, cap_n: int) -> bass.Bass:
    """schedule: list of (expert, tok, offset) chunk descriptors.

    DRAM layouts (host pre-tiled, partition-major):
      xT  [P, CK*cap_n]  chunk k at cols CK*off: row p = concat over cb
                         of x[cb*128+p, tokens_of_chunk]
      w1  [P, E*CK*HS]   expert e at cols e*CK*HS, cb-major
      w2  [P, E*LHK*C]   expert e at cols e*LHK*C, lh-major
      b1  [P, E*LHK]     col g = b1 slice values for h-group g
      yT  [P, CK*cap_n]  same column scheme as xT (co-major per chunk)
    """
    nc = bass.Bass()
    xT = nc.declare_dram_parameter("xT", [P, CK * cap_n], BF16, isOutput=False)
    w1 = nc.declare_dram_parameter("w1", [P, E * CK * HS], BF16, isOutput=False)
    w2 = nc.declare_dram_parameter("w2", [P, E * LHK * C], BF16, isOutput=False)
    b1 = nc.declare_dram_parameter("b1", [P, E * LHK], F32, isOutput=False)
    yT = nc.declare_dram_parameter("yT", [P, CK * cap_n], F32, isOutput=True)

    gelu = mybir.ActivationFunctionType.Gelu

    with tile.TileContext(nc) as tc, ExitStack() as ctx:
        w1p = ctx.enter_context(tc.tile_pool(name="w1p", bufs=E))
        w2p = ctx.enter_context(tc.tile_pool(name="w2p", bufs=E))
        cst = ctx.enter_context(tc.tile_pool(name="cst", bufs=1))
        xp = ctx.enter_context(tc.tile_pool(name="xp", bufs=3))
        hp = ctx.enter_context(tc.tile_pool(name="hp", bufs=2 * LHK))
        op = ctx.enter_context(tc.tile_pool(name="op", bufs=3))
        psA = ctx.enter_context(tc.tile_pool(name="psA", bufs=4, space="PSUM"))
        psB = ctx.enter_context(tc.tile_pool(name="psB", bufs=4, space="PSUM"))

        # Warm the PE HAM clock gate while the first DMAs stream:
        # back-to-back dummy matmuls supply the >=3.4us of sustained PE
        # activity that flips the clock from 1.2 to 2.4 GHz before the
        # real work arrives.
        dummy = cst.tile([P, 512], BF16, tag="dummy")
        nc.gpsimd.memset(dummy[:], 0.0)
        warm = psB.tile([P, 512], F32, tag="psB", name="warm")
        for i in range(34):
            nc.tensor.matmul(warm[:], dummy[:, :P], dummy[:, :512],
                             start=(i == 0), stop=(i == 33))

        # DMA completion tracks emission order (the serial stream
        # saturates per-core HBM BW), so order = criticality: bias
        # (gates the first gelu), expert-0 weights + first chunks, then
        # the rest of the weights; token chunks prefetch just-in-time
        # inside the chunk loop.
        b1_sb = cst.tile([P, E * LHK], F32, tag="b1")
        nc.sync.dma_start(b1_sb[:], b1[:, :])
        # Absorb the bias DMA wait on ScalarE (the activation sync
        # struct fits only one wait and the first gelu needs PE's).
        scr1 = cst.tile([P, 1], F32, tag="scr1")
        nc.scalar.copy(scr1[:], b1_sb[:, 0:1])

        def load_xt(off, tok):
            xt = xp.tile([P, CK, tok], BF16, tag="xt", name=f"xt{off}")
            nc.sync.dma_start(
                xt[:], xT[:, CK * off:CK * (off + tok)]
                .rearrange("p (a m) -> p a m", a=CK))
            return xt

        w1_sb, w2_sb = [], []

        def load_weights(e):
            t1 = w1p.tile([P, CK, HS], BF16, tag="w1", name=f"w1_{e}")
            nc.sync.dma_start(
                t1[:], w1[:, e * CK * HS:(e + 1) * CK * HS]
                .rearrange("p (a m) -> p a m", a=CK))
            w1_sb.append(t1)
            t2 = w2p.tile([P, LHK, C], BF16, tag="w2", name=f"w2_{e}")
            nc.sync.dma_start(
                t2[:], w2[:, e * LHK * C:(e + 1) * LHK * C]
                .rearrange("p (a m) -> p a m", a=LHK))
            w2_sb.append(t2)

        xts = {0: load_xt(schedule[0][2], schedule[0][1])}
        load_weights(0)
        if len(schedule) > 1:
            xts[1] = load_xt(schedule[1][2], schedule[1][1])

        def do_chunk(e, tok, off, xt):
            # phase A: hT[lh] = gelu(w1_slice.T @ xT + b1_slice)
            hts = []
            for lh in range(LHK):
                pa = psA.tile([P, tok], F32, tag="psA", name=f"pa{lh}")
                for cb in range(CK):
                    nc.tensor.matmul(
                        pa[:],
                        w1_sb[e][:, cb, lh * P:(lh + 1) * P],
                        xt[:, cb, :],
                        start=(cb == 0),
                        stop=(cb == CK - 1),
                    )
                ht = hp.tile([P, tok], BF16, tag="ht", name=f"ht{lh}")
                nc.scalar.activation(ht[:], pa[:], gelu,
                                     bias=b1_sb[:, e * LHK + lh:e * LHK + lh + 1])
                hts.append(ht)
            # phase B: yT_partial[co] = w2_slice.T @ hT  (b2 added on host).
            # PSUM drains go to VectorE (ScalarE is busy with gelu); the
            # chunk's output leaves as two DMAs so the final drain and
            # the store overlap better.
            half = CK // 2
            for ho in range(2):
                ot = op.tile([P, half, tok], F32, tag="ot", name=f"ot{ho}")
                for j in range(half):
                    co = ho * half + j
                    pb = psB.tile([P, tok], F32, tag="psB", name=f"pb{co}")
                    for lh in range(LHK):
                        nc.tensor.matmul(
                            pb[:],
                            w2_sb[e][:, lh, co * P:(co + 1) * P],
                            hts[lh][:],
                            start=(lh == 0),
                            stop=(lh == LHK - 1),
                        )
                    nc.vector.tensor_copy(ot[:, j, :], pb[:])
                nc.sync.dma_start(
                    yT[:, CK * off + ho * half * tok:
                       CK * off + (ho + 1) * half * tok]
                    .rearrange("p (a m) -> p a m", a=half), ot[:])

        first_chunk_of = {}
        for k, (e, tok, off) in enumerate(schedule):
            first_chunk_of.setdefault(e, k)
        next_weights = 1
        for k, (e, tok, off) in enumerate(schedule):
            if k + 2 < len(schedule):
                e2, tok2, off2 = schedule[k + 2]
                xts[k + 2] = load_xt(off2, tok2)
            # Stream the next expert's weights one expert-period ahead.
            if next_weights < E and k == first_chunk_of[next_weights - 1]:
                load_weights(next_weights)
                next_weights += 1
            do_chunk(e, tok, off, xts.pop(k))

    _legalize_waits(nc)
    return nc


_NC_CACHE: dict = {}
_LAST_IN_MAPS: list | None = None
_LAST_RESULTS = None


def _routing(xf: np.ndarray, w_gate: np.ndarray):
    logits = xf.astype(np.float64) @ w_gate.astype(np.float64)        # [N, E]
    top_idx = np.argsort(-logits, axis=-1, kind="stable")[:, :TOP_K]  # [N, K]
    top_vals = np.take_along_axis(logits, top_idx, axis=-1)
    ex = np.exp(top_vals - top_vals.max(axis=-1, keepdims=True))
    scores = ex / ex.sum(axis=-1, keepdims=True)                      # [N, K]
    return top_idx, scores


def _ptile(a: np.ndarray) -> np.ndarray:
    """[G*P, M] -> [P, G*M]: row p = concat over g of a[g*128+p, :]."""
    g = a.shape[0] // P
    return np.ascontiguousarray(
        a.reshape(g, P, -1).transpose(1, 0, 2).reshape(P, -1))


def kernel(x, w_gate, w1, b1, w2, b2):
    global _LAST_IN_MAPS, _LAST_RESULTS
    x = np.asarray(x, dtype=np.float32)
    w_gate = np.asarray(w_gate, dtype=np.float32)
    w1 = np.asarray(w1, dtype=np.float32)
    b1 = np.asarray(b1, dtype=np.float32)
    w2 = np.asarray(w2, dtype=np.float32)
    b2 = np.asarray(b2, dtype=np.float32)

    xf = x.reshape(N, C)
    top_idx, scores = _routing(xf, w_gate)

    idx, cw, caps = [], [], []
    for e in range(E):
        hit = top_idx == e                       # [N, K]
        tok = np.nonzero(hit.any(axis=-1))[0]
        idx.append(tok)
        cw.append((scores * hit).sum(axis=-1)[tok].astype(np.float32))
        caps.append(_expert_chunks(len(tok)))

    # Schedule: (expert, chunk_len, token_offset) with experts laid out
    # back to back in one padded token stream.
    schedule, offs = [], []
    off = 0
    for e in range(E):
        offs.append(off)
        for tok in caps[e]:
            schedule.append((e, tok, off))
            off += tok
    cap_n = off

    key = tuple(schedule)
    nc = _NC_CACHE.get(key)
    if nc is None:
        nc = _NC_CACHE[key] = _build_nc(schedule, cap_n)

    # Token stream, pre-tiled per chunk: [P, CK*cap_n] bf16.
    xT_cols = np.zeros((C, cap_n), dtype=ml_dtypes.bfloat16)
    for e in range(E):
        xT_cols[:, offs[e]:offs[e] + len(idx[e])] = \
            xf[idx[e]].T.astype(ml_dtypes.bfloat16)
    xTe = np.empty((P, CK * cap_n), dtype=ml_dtypes.bfloat16)
    for _, tok, off in schedule:
        xTe[:, CK * off:CK * (off + tok)] = _ptile(xT_cols[:, off:off + tok])

    w1_bf = w1.astype(ml_dtypes.bfloat16)   # [E, C, H]
    w2_bf = w2.astype(ml_dtypes.bfloat16)   # [E, H, C]
    in_maps = []
    for c in range(E):
        hs = slice(c * HS, (c + 1) * HS)
        w1c = np.empty((P, E * CK * HS), dtype=ml_dtypes.bfloat16)
        w2c = np.empty((P, E * LHK * C), dtype=ml_dtypes.bfloat16)
        for e in range(E):
            w1c[:, e * CK * HS:(e + 1) * CK * HS] = _ptile(w1_bf[e][:, hs])
            w2c[:, e * LHK * C:(e + 1) * LHK * C] = _ptile(w2_bf[e][hs, :])
        b1c = np.ascontiguousarray(
            b1[:, hs].reshape(E * LHK, P).T)     # [P, E*LHK]
        in_maps.append({"xT": xTe, "w1": w1c, "w2": w2c, "b1": b1c})

    _LAST_IN_MAPS = in_maps
    res = run_bass_kernel_spmd(nc, in_maps, list(range(E)))
    _LAST_RESULTS = res

    # Combine: sum the 8 H-slice partials, de-tile, add b2, apply gate
    # weights, scatter-add back to token order.
    Y2 = res.results[0]["yT"].astype(np.float32)
    for c in range(1, E):
        Y2 += res.results[c]["yT"]
    Y = np.empty((cap_n, C), dtype=np.float32)   # token-major
    for _, tok, off in schedule:
        Y[off:off + tok] = (Y2[:, CK * off:CK * (off + tok)]
                            .reshape(P, CK, tok).transpose(1, 0, 2)
                            .reshape(C, tok).T)
    out = np.zeros((N, C), dtype=np.float32)
    for e in range(E):
        ne = len(idx[e])
        out[idx[e]] += cw[e][:, None] * (Y[offs[e]:offs[e] + ne] + b2[e])
    return out.reshape(B, T, C)


# revision 33
# speedup vs baseline: 1.2235x; 1.0002x over previous
"""Trainium2 Bass kernel for an 8-expert top-2 MoE block (B=4, T=2048, C=1024, H=4C).

Strategy (hidden-dim tensor-parallel over all experts):
  - Host computes the gate (logits -> top-2 -> softmax) and gathers each
    expert's routed tokens into one padded token stream (the "all-to-all
    dispatch", done host-side as the sharding step).
  - Each of the 8 NeuronCores holds a 512-wide slice of the hidden (H)
    dimension of ALL 8 experts' weights (16MB bf16 total, same as one
    expert's full weights) and runs gelu(x @ w1_slice + b1_slice) @
    w2_slice over EVERY routed token. This is perfectly load-balanced
    regardless of routing skew, unlike expert-parallel where every core
    pays for the hottest expert.
  - The per-expert token stream is chunked into variable-size chunks
    (<=512 tokens, PSUM bank limit; >=192 so LDWEIGHTS stays hidden
    behind the moving operand), so padding waste is <0.5%.
  - All device tensors are pre-tiled on the host into partition-major
    [128, *] layouts with >=8KB contiguous per partition, so every
    weight block / token chunk / output chunk moves as ONE efficient
    DMA (the serial DMA stream is otherwise instruction-bound).
  - Host sums the 8 partial outputs (the H-dimension reduction), adds
    b2, applies the top-2 combine weights, and scatter-adds into the
    full [B, T, C] output (the "combine"/unshard).

All matmuls are bf16 with fp32 PSUM accumulation; rel err vs the fp32
reference is ~3e-3.
"""

import sys

for _p in ("/opt/trn_rl_repo", "/root/.axon_site/_ro/trn_rl_repo"):
    if _p not in sys.path:
        sys.path.insert(0, _p)

from contextlib import ExitStack

import ml_dtypes
import numpy as np

import bass_rust
import concourse.bass as bass
import concourse.mybir as mybir
import concourse.tile as tile
from concourse.bass_utils import run_bass_kernel_spmd

B, T, C, E = 4, 2048, 1024, 8
H = 4 * C
N = B * T
TOP_K = 2
P = 128            # partitions
HS = H // 8        # per-core hidden slice (512)
CK = C // P        # 8 contraction chunks for x @ w1
LHK = HS // P      # 4 local h-groups per chunk
TOK = 512          # max token chunk (PSUM bank = 512 fp32)
MIN_TAIL = 192     # min tail chunk so LDWEIGHTS stays hidden

BF16 = mybir.dt.bfloat16
F32 = mybir.dt.float32


def _legalize_waits(nc: "bass.Bass") -> None:
    """Split multi-wait instructions into standalone EventSemaphore waits.

    The walrus build here accepts at most one sync-wait command per
    instruction (setupSyncWait "Too many sync wait commands"), but Tile
    attaches every outstanding dependency to the consuming instruction.
    Hoist all but the last wait onto same-engine EventSemaphore
    instructions placed immediately before the consumer: the engine's
    sequencer processes them in order, so the dependency still holds.
    """

    def fix_block(bb):
        out = []
        for inst in bb.instructions:
            si = inst.sync_info
            if si is not None and len(si.on_wait) > 1:
                waits = list(si.on_wait)
                for k, w in enumerate(waits[:-1]):
                    ev = bass_rust.InstEventSemaphore(
                        name=f"{inst.name}-lw{k}", ins=[], outs=[],
                        engine=inst.engine,
                    )
                    ev.sync_info = bass_rust.SyncInfo(on_wait=[w], on_update=[])
                    out.append(ev)
                inst.sync_info = bass_rust.SyncInfo(
                    on_wait=[waits[-1]], on_update=list(si.on_update)
                )
            out.append(inst)
        bb.instructions = out
        for sub in getattr(bb, "blocks", []) or []:
            fix_block(sub)

    for fn in nc.m.functions:
        for bb in fn.blocks:
            fix_block(bb)


def _ceil16(v: int) -> int:
    return -(-v // 16) * 16


def _expert_chunks(count: int) -> list[int]:
    """Chunk sizes for one expert's token stream: full 512s plus one or
    two tails in [MIN_TAIL, 512], padded to a multiple of 16."""
    if count <= 0:
        return []
    k, r = divmod(count, TOK)
    if r == 0:
        return [TOK] * k
    if r >= MIN_TAIL or k == 0:
        return [TOK] * k + [max(_ceil16(r), MIN_TAIL if k else 16)]
    # Small remainder: borrow one full chunk and split into two tails.
    r += TOK
    t1 = max(_ceil16(r // 2), MIN_TAIL)
    t2 = max(_ceil16(r - t1), MIN_TAIL)
    return [TOK] * (k - 1) + [t1, t2]


def _build_nc(schedule, cap_n: int) -> bass.Bass:
    """schedule: list of (expert, tok, offset) chunk descriptors.

    DRAM layouts (host pre-tiled, partition-major):
      xT  [P, CK*cap_n]  chunk k at cols CK*off: row p = concat over cb
                         of x[cb*128+p, tokens_of_chunk]
      w1  [P, E*CK*HS]   expert e at cols e*CK*HS, cb-major
      w2  [P, E*LHK*C]   expert e at cols e*LHK*C, lh-major
      b1  [P, E*LHK]     col g = b1 slice values for h-group g
      yT  [P, CK*cap_n]  same column scheme as xT (co-major per chunk)
    """
    nc = bass.Bass()
    xT = nc.declare_dram_parameter("xT", [P, CK * cap_n], BF16, isOutput=False)
    w1 = nc.declare_dram_parameter("w1", [P, E * CK * HS], BF16, isOutput=False)
    w2 = nc.declare_dram_parameter("w2", [P, E * LHK * C], BF16, isOutput=False)
    b1 = nc.declare_dram_parameter("b1", [P, E * LHK], F32, isOutput=False)
    yT = nc.declare_dram_parameter("yT", [P, CK * cap_n], F32, isOutput=True)

    gelu = mybir.ActivationFunctionType.Gelu

    with tile.TileContext(nc) as tc, ExitStack() as ctx:
        w1p = ctx.enter_context(tc.tile_pool(name="w1p", bufs=E))
        w2p = ctx.enter_context(tc.tile_pool(name="w2p", bufs=E))
        cst = ctx.enter_context(tc.tile_pool(name="cst", bufs=1))
        xp = ctx.enter_context(tc.tile_pool(name="xp", bufs=3))
        hp = ctx.enter_context(tc.tile_pool(name="hp", bufs=2 * LHK))
        op = ctx.enter_context(tc.tile_pool(name="op", bufs=3))
        psA = ctx.enter_context(tc.tile_pool(name="psA", bufs=4, space="PSUM"))
        psB = ctx.enter_context(tc.tile_pool(name="psB", bufs=4, space="PSUM"))

        # Warm the PE HAM clock gate while the first DMAs stream:
        # back-to-back dummy matmuls supply the >=3.4us of sustained PE
        # activity that flips the clock from 1.2 to 2.4 GHz before the
        # real work arrives.
        dummy = cst.tile([P, 512], BF16, tag="dummy")
        nc.gpsimd.memset(dummy[:], 0.0)
        warm = psB.tile([P, 512], F32, tag="psB", name="warm")
        for i in range(34):
            nc.tensor.matmul(warm[:], dummy[:, :P], dummy[:, :512],
                             start=(i == 0), stop=(i == 33))

        # DMA completion tracks emission order (the serial stream
        # saturates per-core HBM BW), so order = criticality: bias
        # (gates the first gelu), expert-0 weights + first chunks, then
        # the rest of the weights; token chunks prefetch just-in-time
        # inside the chunk loop.
        b1_sb = cst.tile([P, E * LHK], F32, tag="b1")
        nc.sync.dma_start(b1_sb[:], b1[:, :])
        # Absorb the bias DMA wait on ScalarE (the activation sync
        # struct fits only one wait and the first gelu needs PE's).
        scr1 = cst.tile([P, 1], F32, tag="scr1")
        nc.scalar.copy(scr1[:], b1_sb[:, 0:1])

        def load_xt(off, tok):
            xt = xp.tile([P, CK, tok], BF16, tag="xt", name=f"xt{off}")
            nc.sync.dma_start(
                xt[:], xT[:, CK * off:CK * (off + tok)]
                .rearrange("p (a m) -> p a m", a=CK))
            return xt

        w1_sb, w2_sb = [], []

        def load_weights(e):
            t1 = w1p.tile([P, CK, HS], BF16, tag="w1", name=f"w1_{e}")
            nc.sync.dma_start(
                t1[:], w1[:, e * CK * HS:(e + 1) * CK * HS]
                .rearrange("p (a m) -> p a m", a=CK))
            w1_sb.append(t1)
            t2 = w2p.tile([P, LHK, C], BF16, tag="w2", name=f"w2_{e}")
            nc.sync.dma_start(
                t2[:], w2[:, e * LHK * C:(e + 1) * LHK * C]
                .rearrange("p (a m) -> p a m", a=LHK))
            w2_sb.append(t2)

        xts = {0: load_xt(schedule[0][2], schedule[0][1])}
        load_weights(0)
        if len(schedule) > 1:
            xts[1] = load_xt(schedule[1][2], schedule[1][1])

        def do_chunk(e, tok, off, xt):
            # phase A: hT[lh] = gelu(w1_slice.T @ xT + b1_slice)
            hts = []
            for lh in range(LHK):
                pa = psA.tile([P, tok], F32, tag="psA", name=f"pa{lh}")
                for cb in range(CK):
                    nc.tensor.matmul(
                        pa[:],
                        w1_sb[e][:, cb, lh * P:(lh + 1) * P],
                        xt[:, cb, :],
                        start=(cb == 0),
                        stop=(cb == CK - 1),
                    )
                ht = hp.tile([P, tok], BF16, tag="ht", name=f"ht{lh}")
                nc.scalar.activation(ht[:], pa[:], gelu,
                                     bias=b1_sb[:, e * LHK + lh:e * LHK + lh + 1])
                hts.append(ht)
            # phase B: yT_partial[co] = w2_slice.T @ hT  (b2 added on host).
            # PSUM drains go to VectorE (ScalarE is busy with gelu); the
            # chunk's output leaves as two DMAs so the final drain and
            # the store overlap better.
            half = CK // 2
            for ho in range(2):
                ot = op.tile([P, half, tok], F32, tag="ot", name=f"ot{ho}")
                for j in range(half):
                    co = ho * half + j
                    pb = psB.tile([P, tok], F32, tag="psB", name=f"pb{co}")
                    for lh in range(LHK):
                        nc.tensor.matmul(
                            pb[:],
                            w2_sb[e][:, lh, co * P:(co + 1) * P],
                            hts[lh][:],
                            start=(lh == 0),
                            stop=(lh == LHK - 1),
                        )
                    nc.vector.tensor_copy(ot[:, j, :], pb[:])
                nc.sync.dma_start(
                    yT[:, CK * off + ho * half * tok:
                       CK * off + (ho + 1) * half * tok]
                    .rearrange("p (a m) -> p a m", a=half), ot[:])

        for k, (e, tok, off) in enumerate(schedule):
            if k + 2 < len(schedule):
                e2, tok2, off2 = schedule[k + 2]
                xts[k + 2] = load_xt(off2, tok2)
            # Stream weights just-in-time, one expert ahead of compute.
            while len(w1_sb) <= min(e + 1, E - 1):
                load_weights(len(w1_sb))
            do_chunk(e, tok, off, xts.pop(k))

    _legalize_waits(nc)
    return nc


_NC_CACHE: dict = {}
_LAST_IN_MAPS: list | None = None
_LAST_RESULTS = None


def _routing(xf: np.ndarray, w_gate: np.ndarray):
    logits = xf.astype(np.float64) @ w_gate.astype(np.float64)        # [N, E]
    top_idx = np.argsort(-logits, axis=-1, kind="stable")[:, :TOP_K]  # [N, K]
    top_vals = np.take_along_axis(logits, top_idx, axis=-1)
    ex = np.exp(top_vals - top_vals.max(axis=-1, keepdims=True))
    scores = ex / ex.sum(axis=-1, keepdims=True)                      # [N, K]
    return top_idx, scores


def _ptile(a: np.ndarray) -> np.ndarray:
    """[G*P, M] -> [P, G*M]: row p = concat over g of a[g*128+p, :]."""
    g = a.shape[0] // P
    return np.ascontiguousarray(
        a.reshape(g, P, -1).transpose(1, 0, 2).reshape(P, -1))


def kernel(x, w_gate, w1, b1, w2, b2):
    global _LAST_IN_MAPS, _LAST_RESULTS
    x = np.asarray(x, dtype=np.float32)
    w_gate = np.asarray(w_gate, dtype=np.float32)
    w1 = np.asarray(w1, dtype=np.float32)
    b1 = np.asarray(b1, dtype=np.float32)
    w2 = np.asarray(w2, dtype=np.float32)
    b2 = np.asarray(b2, dtype=np.float32)

    xf = x.reshape(N, C)
    top_idx, scores = _routing(xf, w_gate)

    idx, cw, caps = [], [], []
    for e in range(E):
        hit = top_idx == e                       # [N, K]
        tok = np.nonzero(hit.any(axis=-1))[0]
        idx.append(tok)
        cw.append((scores * hit).sum(axis=-1)[tok].astype(np.float32))
        caps.append(_expert_chunks(len(tok)))

    # Schedule: (expert, chunk_len, token_offset) with experts laid out
    # back to back in one padded token stream.
    schedule, offs = [], []
    off = 0
    for e in range(E):
        offs.append(off)
        for tok in caps[e]:
            schedule.append((e, tok, off))
            off += tok
    cap_n = off

    key = tuple(schedule)
    nc = _NC_CACHE.get(key)
    if nc is None:
        nc = _NC_CACHE[key] = _build_nc(schedule, cap_n)

    # Token stream, pre-tiled per chunk: [P, CK*cap_n] bf16.
    xT_cols = np.zeros((C, cap_n), dtype=ml_dtypes.bfloat16)
    for e in range(E):
        xT_cols[:, offs[e]:offs[e] + len(idx[e])] = \
            xf[idx[e]].T.astype(ml_dtypes.bfloat16)
    xTe = np.empty((P, CK * cap_n), dtype=ml_dtypes.bfloat16)
    for _, tok, off in schedule:
        xTe[:, CK * off:CK * (off + tok)] = _ptile(xT_cols[:, off:off + tok])

    w1_bf = w1.astype(ml_dtypes.bfloat16)   # [E, C, H]
    w2_bf = w2.astype(ml_dtypes.bfloat16)   # [E, H, C]
    in_maps = []
    for c in range(E):
        hs = slice(c * HS, (c + 1) * HS)
        w1c = np.empty((P, E * CK * HS), dtype=ml_dtypes.bfloat16)
        w2c = np.empty((P, E * LHK * C), dtype=ml_dtypes.bfloat16)
        for e in range(E):
            w1c[:, e * CK * HS:(e + 1) * CK * HS] = _ptile(w1_bf[e][:, hs])
            w2c[:, e * LHK * C:(e + 1) * LHK * C] = _ptile(w2_bf[e][hs, :])
        b1c = np.ascontiguousarray(
            b1[:, hs].reshape(E * LHK, P).T)     # [P, E*LHK]
        in_maps.append({"xT": xTe, "w1": w1c, "w2": w2c, "b1": b1c})

    _LAST_IN_MAPS = in_maps
    res = run_bass_kernel_spmd(nc, in_maps, list(range(E)))
    _LAST_RESULTS = res

    # Combine: sum the 8 H-slice partials, de-tile, add b2, apply gate
    # weights, scatter-add back to token order.
    Y2 = res.results[0]["yT"].astype(np.float32)
    for c in range(1, E):
        Y2 += res.results[c]["yT"]
    Y = np.empty((cap_n, C), dtype=np.float32)   # token-major
    for _, tok, off in schedule:
        Y[off:off + tok] = (Y2[:, CK * off:CK * (off + tok)]
                            .reshape(P, CK, tok).transpose(1, 0, 2)
                            .reshape(C, tok).T)
    out = np.zeros((N, C), dtype=np.float32)
    for e in range(E):
        ne = len(idx[e])
        out[idx[e]] += cw[e][:, None] * (Y[offs[e]:offs[e] + ne] + b2[e])
    return out.reshape(B, T, C)


# revision 35
# speedup vs baseline: 1.2256x; 1.0017x over previous
"""Trainium2 Bass kernel for an 8-expert top-2 MoE block (B=4, T=2048, C=1024, H=4C).

Strategy (hidden-dim tensor-parallel over all experts):
  - Host computes the gate (logits -> top-2 -> softmax) and gathers each
    expert's routed tokens into one padded token stream (the "all-to-all
    dispatch", done host-side as the sharding step).
  - Each of the 8 NeuronCores holds a 512-wide slice of the hidden (H)
    dimension of ALL 8 experts' weights (16MB bf16 total, same as one
    expert's full weights) and runs gelu(x @ w1_slice + b1_slice) @
    w2_slice over EVERY routed token. This is perfectly load-balanced
    regardless of routing skew, unlike expert-parallel where every core
    pays for the hottest expert.
  - The per-expert token stream is chunked into variable-size chunks
    (<=512 tokens, PSUM bank limit; >=192 so LDWEIGHTS stays hidden
    behind the moving operand), so padding waste is <0.5%.
  - All device tensors are pre-tiled on the host into partition-major
    [128, *] layouts with >=8KB contiguous per partition, so every
    weight block / token chunk / output chunk moves as ONE efficient
    DMA (the serial DMA stream is otherwise instruction-bound).
  - Host sums the 8 partial outputs (the H-dimension reduction), adds
    b2, applies the top-2 combine weights, and scatter-adds into the
    full [B, T, C] output (the "combine"/unshard).

All matmuls are bf16 with fp32 PSUM accumulation; rel err vs the fp32
reference is ~3e-3.
"""

import sys

for _p in ("/opt/trn_rl_repo", "/root/.axon_site/_ro/trn_rl_repo"):
    if _p not in sys.path:
        sys.path.insert(0, _p)

from contextlib import ExitStack

import ml_dtypes
import numpy as np

import bass_rust
import concourse.bass as bass
import concourse.mybir as mybir
import concourse.tile as tile
from concourse.bass_utils import run_bass_kernel_spmd

B, T, C, E = 4, 2048, 1024, 8
H = 4 * C
N = B * T
TOP_K = 2
P = 128            # partitions
HS = H // 8        # per-core hidden slice (512)
CK = C // P        # 8 contraction chunks for x @ w1
LHK = HS // P      # 4 local h-groups per chunk
TOK = 512          # max token chunk (PSUM bank = 512 fp32)
MIN_TAIL = 192     # min tail chunk so LDWEIGHTS stays hidden

BF16 = mybir.dt.bfloat16
F32 = mybir.dt.float32


def _legalize_waits(nc: "bass.Bass") -> None:
    """Split multi-wait instructions into standalone EventSemaphore waits.

    The walrus build here accepts at most one sync-wait command per
    instruction (setupSyncWait "Too many sync wait commands"), but Tile
    attaches every outstanding dependency to the consuming instruction.
    Hoist all but the last wait onto same-engine EventSemaphore
    instructions placed immediately before the consumer: the engine's
    sequencer processes them in order, so the dependency still holds.
    """

    def fix_block(bb):
        out = []
        for inst in bb.instructions:
            si = inst.sync_info
            if si is not None and len(si.on_wait) > 1:
                waits = list(si.on_wait)
                for k, w in enumerate(waits[:-1]):
                    ev = bass_rust.InstEventSemaphore(
                        name=f"{inst.name}-lw{k}", ins=[], outs=[],
                        engine=inst.engine,
                    )
                    ev.sync_info = bass_rust.SyncInfo(on_wait=[w], on_update=[])
                    out.append(ev)
                inst.sync_info = bass_rust.SyncInfo(
                    on_wait=[waits[-1]], on_update=list(si.on_update)
                )
            out.append(inst)
        bb.instructions = out
        for sub in getattr(bb, "blocks", []) or []:
            fix_block(sub)

    for fn in nc.m.functions:
        for bb in fn.blocks:
            fix_block(bb)


def _ceil16(v: int) -> int:
    return -(-v // 16) * 16


def _expert_chunks(count: int) -> list[int]:
    """Chunk sizes for one expert's token stream: full 512s plus one or
    two tails in [MIN_TAIL, 512], padded to a multiple of 16."""
    if count <= 0:
        return []
    k, r = divmod(count, TOK)
    if r == 0:
        return [TOK] * k
    if r >= MIN_TAIL or k == 0:
        return [TOK] * k + [max(_ceil16(r), MIN_TAIL if k else 16)]
    # Small remainder: borrow one full chunk and split into two tails.
    r += TOK
    t1 = max(_ceil16(r // 2), MIN_TAIL)
    t2 = max(_ceil16(r - t1), MIN_TAIL)
    return [TOK] * (k - 1) + [t1, t2]


def _build_nc(schedule, cap_n: int) -> bass.Bass:
    """schedule: list of (expert, tok, offset) chunk descriptors.

    DRAM layouts (host pre-tiled, partition-major):
      xT  [P, CK*cap_n]  chunk k at cols CK*off: row p = concat over cb
                         of x[cb*128+p, tokens_of_chunk]
      w1  [P, E*CK*HS]   expert e at cols e*CK*HS, cb-major
      w2  [P, E*LHK*C]   expert e at cols e*LHK*C, lh-major
      b1  [P, E*LHK]     col g = b1 slice values for h-group g
      yT  [P, CK*cap_n]  same column scheme as xT (co-major per chunk)
    """
    nc = bass.Bass()
    xT = nc.declare_dram_parameter("xT", [P, CK * cap_n], BF16, isOutput=False)
    w1 = nc.declare_dram_parameter("w1", [P, E * CK * HS], BF16, isOutput=False)
    w2 = nc.declare_dram_parameter("w2", [P, E * LHK * C], BF16, isOutput=False)
    b1 = nc.declare_dram_parameter("b1", [P, E * LHK], F32, isOutput=False)
    yT = nc.declare_dram_parameter("yT", [P, CK * cap_n], F32, isOutput=True)

    gelu = mybir.ActivationFunctionType.Gelu

    with tile.TileContext(nc) as tc, ExitStack() as ctx:
        w1p = ctx.enter_context(tc.tile_pool(name="w1p", bufs=E))
        w2p = ctx.enter_context(tc.tile_pool(name="w2p", bufs=E))
        cst = ctx.enter_context(tc.tile_pool(name="cst", bufs=1))
        xp = ctx.enter_context(tc.tile_pool(name="xp", bufs=3))
        hp = ctx.enter_context(tc.tile_pool(name="hp", bufs=2 * LHK))
        op = ctx.enter_context(tc.tile_pool(name="op", bufs=3))
        psA = ctx.enter_context(tc.tile_pool(name="psA", bufs=4, space="PSUM"))
        psB = ctx.enter_context(tc.tile_pool(name="psB", bufs=4, space="PSUM"))

        # Warm the PE HAM clock gate while the first DMAs stream:
        # back-to-back dummy matmuls supply the >=3.4us of sustained PE
        # activity that flips the clock from 1.2 to 2.4 GHz before the
        # real work arrives.
        dummy = cst.tile([P, 512], BF16, tag="dummy")
        nc.gpsimd.memset(dummy[:], 0.0)
        warm = psB.tile([P, 512], F32, tag="psB", name="warm")
        for i in range(26):
            nc.tensor.matmul(warm[:], dummy[:, :P], dummy[:, :512],
                             start=(i == 0), stop=(i == 25))

        # DMA completion tracks emission order (the serial stream
        # saturates per-core HBM BW), so order = criticality: bias
        # (gates the first gelu), expert-0 weights + first chunks, then
        # the rest of the weights; token chunks prefetch just-in-time
        # inside the chunk loop.
        b1_sb = cst.tile([P, E * LHK], F32, tag="b1")
        nc.sync.dma_start(b1_sb[:], b1[:, :])
        # Absorb the bias DMA wait on ScalarE (the activation sync
        # struct fits only one wait and the first gelu needs PE's).
        scr1 = cst.tile([P, 1], F32, tag="scr1")
        nc.scalar.copy(scr1[:], b1_sb[:, 0:1])

        def load_xt(off, tok):
            xt = xp.tile([P, CK, tok], BF16, tag="xt", name=f"xt{off}")
            nc.sync.dma_start(
                xt[:], xT[:, CK * off:CK * (off + tok)]
                .rearrange("p (a m) -> p a m", a=CK))
            return xt

        w1_sb, w2_sb = [], []

        def load_weights(e):
            t1 = w1p.tile([P, CK, HS], BF16, tag="w1", name=f"w1_{e}")
            nc.sync.dma_start(
                t1[:], w1[:, e * CK * HS:(e + 1) * CK * HS]
                .rearrange("p (a m) -> p a m", a=CK))
            w1_sb.append(t1)
            t2 = w2p.tile([P, LHK, C], BF16, tag="w2", name=f"w2_{e}")
            nc.sync.dma_start(
                t2[:], w2[:, e * LHK * C:(e + 1) * LHK * C]
                .rearrange("p (a m) -> p a m", a=LHK))
            w2_sb.append(t2)

        xts = {0: load_xt(schedule[0][2], schedule[0][1])}
        load_weights(0)
        if len(schedule) > 1:
            xts[1] = load_xt(schedule[1][2], schedule[1][1])

        def do_chunk(e, tok, off, xt):
            # phase A: hT[lh] = gelu(w1_slice.T @ xT + b1_slice)
            hts = []
            for lh in range(LHK):
                pa = psA.tile([P, tok], F32, tag="psA", name=f"pa{lh}")
                for cb in range(CK):
                    nc.tensor.matmul(
                        pa[:],
                        w1_sb[e][:, cb, lh * P:(lh + 1) * P],
                        xt[:, cb, :],
                        start=(cb == 0),
                        stop=(cb == CK - 1),
                    )
                ht = hp.tile([P, tok], BF16, tag="ht", name=f"ht{lh}")
                nc.scalar.activation(ht[:], pa[:], gelu,
                                     bias=b1_sb[:, e * LHK + lh:e * LHK + lh + 1])
                hts.append(ht)
            # phase B: yT_partial[co] = w2_slice.T @ hT  (b2 added on host).
            # PSUM drains go to VectorE (ScalarE is busy with gelu); the
            # chunk's output leaves as two DMAs so the final drain and
            # the store overlap better.
            half = CK // 2
            for ho in range(2):
                ot = op.tile([P, half, tok], F32, tag="ot", name=f"ot{ho}")
                for j in range(half):
                    co = ho * half + j
                    pb = psB.tile([P, tok], F32, tag="psB", name=f"pb{co}")
                    for lh in range(LHK):
                        nc.tensor.matmul(
                            pb[:],
                            w2_sb[e][:, lh, co * P:(co + 1) * P],
                            hts[lh][:],
                            start=(lh == 0),
                            stop=(lh == LHK - 1),
                        )
                    nc.vector.tensor_copy(ot[:, j, :], pb[:])
                nc.sync.dma_start(
                    yT[:, CK * off + ho * half * tok:
                       CK * off + (ho + 1) * half * tok]
                    .rearrange("p (a m) -> p a m", a=half), ot[:])

        for k, (e, tok, off) in enumerate(schedule):
            if k + 2 < len(schedule):
                e2, tok2, off2 = schedule[k + 2]
                xts[k + 2] = load_xt(off2, tok2)
            # Stream weights just-in-time, one expert ahead of compute.
            while len(w1_sb) <= min(e + 1, E - 1):
                load_weights(len(w1_sb))
            do_chunk(e, tok, off, xts.pop(k))

    _legalize_waits(nc)
    return nc


_NC_CACHE: dict = {}
_LAST_IN_MAPS: list | None = None
_LAST_RESULTS = None


def _routing(xf: np.ndarray, w_gate: np.ndarray):
    logits = xf.astype(np.float64) @ w_gate.astype(np.float64)        # [N, E]
    top_idx = np.argsort(-logits, axis=-1, kind="stable")[:, :TOP_K]  # [N, K]
    top_vals = np.take_along_axis(logits, top_idx, axis=-1)
    ex = np.exp(top_vals - top_vals.max(axis=-1, keepdims=True))
    scores = ex / ex.sum(axis=-1, keepdims=True)                      # [N, K]
    return top_idx, scores


def _ptile(a: np.ndarray) -> np.ndarray:
    """[G*P, M] -> [P, G*M]: row p = concat over g of a[g*128+p, :]."""
    g = a.shape[0] // P
    return np.ascontiguousarray(
        a.reshape(g, P, -1).transpose(1, 0, 2).reshape(P, -1))


def kernel(x, w_gate, w1, b1, w2, b2):
    global _LAST_IN_MAPS, _LAST_RESULTS
    x = np.asarray(x, dtype=np.float32)
    w_gate = np.asarray(w_gate, dtype=np.float32)
    w1 = np.asarray(w1, dtype=np.float32)
    b1 = np.asarray(b1, dtype=np.float32)
    w2 = np.asarray(w2, dtype=np.float32)
    b2 = np.asarray(b2, dtype=np.float32)

    xf = x.reshape(N, C)
    top_idx, scores = _routing(xf, w_gate)

    idx, cw, caps = [], [], []
    for e in range(E):
        hit = top_idx == e                       # [N, K]
        tok = np.nonzero(hit.any(axis=-1))[0]
        idx.append(tok)
        cw.append((scores * hit).sum(axis=-1)[tok].astype(np.float32))
        caps.append(_expert_chunks(len(tok)))

    # Schedule: (expert, chunk_len, token_offset) with experts laid out
    # back to back in one padded token stream. Expert 0 leads with its
    # smallest chunk (first xt+w1 DMAs complete sooner -> earlier PE
    # start); the last expert ends with its smallest (shorter final
    # store before the drain barrier).
    if caps and caps[0]:
        caps[0] = sorted(caps[0])
    if caps and caps[-1]:
        caps[-1] = sorted(caps[-1], reverse=True)
    schedule, offs = [], []
    off = 0
    for e in range(E):
        offs.append(off)
        for tok in caps[e]:
            schedule.append((e, tok, off))
            off += tok
    cap_n = off

    key = tuple(schedule)
    nc = _NC_CACHE.get(key)
    if nc is None:
        nc = _NC_CACHE[key] = _build_nc(schedule, cap_n)

    # Token stream, pre-tiled per chunk: [P, CK*cap_n] bf16.
    xT_cols = np.zeros((C, cap_n), dtype=ml_dtypes.bfloat16)
    for e in range(E):
        xT_cols[:, offs[e]:offs[e] + len(idx[e])] = \
            xf[idx[e]].T.astype(ml_dtypes.bfloat16)
    xTe = np.empty((P, CK * cap_n), dtype=ml_dtypes.bfloat16)
    for _, tok, off in schedule:
        xTe[:, CK * off:CK * (off + tok)] = _ptile(xT_cols[:, off:off + tok])

    w1_bf = w1.astype(ml_dtypes.bfloat16)   # [E, C, H]
    w2_bf = w2.astype(ml_dtypes.bfloat16)   # [E, H, C]
    in_maps = []
    for c in range(E):
        hs = slice(c * HS, (c + 1) * HS)
        w1c = np.empty((P, E * CK * HS), dtype=ml_dtypes.bfloat16)
        w2c = np.empty((P, E * LHK * C), dtype=ml_dtypes.bfloat16)
        for e in range(E):
            w1c[:, e * CK * HS:(e + 1) * CK * HS] = _ptile(w1_bf[e][:, hs])
            w2c[:, e * LHK * C:(e + 1) * LHK * C] = _ptile(w2_bf[e][hs, :])
        b1c = np.ascontiguousarray(
            b1[:, hs].reshape(E * LHK, P).T)     # [P, E*LHK]
        in_maps.append({"xT": xTe, "w1": w1c, "w2": w2c, "b1": b1c})

    _LAST_IN_MAPS = in_maps
    res = run_bass_kernel_spmd(nc, in_maps, list(range(E)))
    _LAST_RESULTS = res

    # Combine: sum the 8 H-slice partials, de-tile, add b2, apply gate
    # weights, scatter-add back to token order.
    Y2 = res.results[0]["yT"].astype(np.float32)
    for c in range(1, E):
        Y2 += res.results[c]["yT"]
    Y = np.empty((cap_n, C), dtype=np.float32)   # token-major
    for _, tok, off in schedule:
        Y[off:off + tok] = (Y2[:, CK * off:CK * (off + tok)]
                            .reshape(P, CK, tok).transpose(1, 0, 2)
                            .reshape(C, tok).T)
    out = np.zeros((N, C), dtype=np.float32)
    for e in range(E):
        ne = len(idx[e])
        out[idx[e]] += cw[e][:, None] * (Y[offs[e]:offs[e] + ne] + b2[e])
    return out.reshape(B, T, C)
